# revision 36
# baseline (speedup 1.0000x reference)
"""Trainium2 Bass kernel for nn_Decoder (GRU decoder over padded sequences).

Computation (per sample):
  emb = message[:, :T-1] @ W_emb.T + b_emb            (folded into W_c on host)
  xs  = [init_emb, emb]                                (step 0 folded into h1 const)
  GRU over T steps, gather h at lengths-1              (freeze trick: z := 1 past len)
  out = sigmoid(elu(h @ W1.T + b1) @ W2.T + b2)

Sharding: batch data-parallel over 8 cores, host-side length-sort (stratified
round-robin across cores) so each group only runs to its max length.

v2 layout: two 512-sample strata are packed on the partition axis (stratum A
on partitions 0..63, B on 64..127) via block-diagonal weight matrices, so
every elementwise op runs with all 128 partitions at free-dim 512 (DVE/ACT
cost scales with free size only). Sigmoids are computed as
0.5*tanh(x/2)+0.5 so the whole kernel lives in one ACT table (Tanh+Exp, no
table swaps) and r+z share a single [128,1024] tanh. Gate biases ride a
constant ones-row in the streamed X tile; b_hn is applied by the DVE
scalar_tensor_tensor that forms r*(hn+b_hn). The xn + r*hn sum is
accumulated in PSUM by an identity matmul on the TensorEngine.

v4: sigmoids via tanh: rz gates = 0.5*tanh(x/2)+0.5 (ACT scale=0.5 pre-affine,
+1 tensor_scalar on gpsimd/DVE), W2 output stored as tanh((x+b2)/2) with the
(t+1)/2 affine done on host in f32 (free + more accurate). Direct AF.Sigmoid
lives on a different ACT table than AF.Exp (ELU), and the scheduler was
interleaving head ELU exps into the GRU tail: 18 ping-pong ACT_TABLE_LOADs
(1283ns each) sat on the ACT queue right where the tail chain is
latency-bound. Single-table: 1 load total, -26us + better accuracy (6.8e-4).

v3 HAM notes (measured on HW): the PE clock gate (HAM) keeps the array at
1.2GHz cold unless a 4096-cycle window is ~fully busy with array STREAMING
(LDWEIGHTS time does not count). Steady GRU = 80% array-busy (each of the 7
stationary changes/group-step exposes ~107ns LDWEIGHTS) so it never warms on
its own; the 50-matmul same-stationary warmup burst at t=0 streams 100%-dense
during the ~20us init-DMA window and fires K=8/8 for the first ~2 waves.
v4: single ACT table (see below) changed the balance — 3 mid-GRU re-warm
bursts at waves 6/10/14 now NET HELP (-28us; warm stretches survive 30-75us).
Denser bursts (10 of them) measured WORSE (+20us of redundant streaming).
Still not helping: forced same-stationary pair-batching via tile_wait_until
(raises density to 89% and SHORT does re-fire, but at 2.4GHz array-busy drops
to 82% and Activity_MID immediately re-throttles; the reorder also lengthens
the pair serial chain, hurting the tail), T2_VIA_PE=False (+100us DVE).
Head weights are DMA'd after the GRU loop so wave-1 X wins the DMA queue.
Beware: the chip drops to a ~2.0GHz PLL state (P0) after sustained load —
run-to-run exec varies 590-730ns; compare only like-state runs (cold-matmul
duration 605-612 = fast state, ~730 = P0).
"""

import sys

sys.path.insert(0, "/opt/trn_rl_repo")

import numpy as np
import ml_dtypes

import concourse.bacc as bacc
import concourse.mybir as mybir
import concourse.tile as tile
from concourse.bass_utils import run_bass_kernel_spmd

B, T, V, E, H, FC, OUT = 65536, 30, 21, 32, 64, 256, 784
NCORES = 8
BC = B // NCORES          # 8192 samples per core
SW = 512                  # stratum width (free dim of GRU ops)
G = 8                     # groups per core; group = 2 strata of SW samples
NSG = G // 2              # super-groups for the MLP head (free dim 1024)
FREEZE = 40.0             # z-gate preactivation offset for finished samples
XB = 4                    # GRU steps per streamed X block
OTILES = (OUT + 127) // 128  # 7 output row tiles
KR = 64                   # X tile partition rows (A block 0..31, B block 32..63)

T2_VIA_PE = True          # accumulate t1 into xn PSUM with identity matmul
REWARM_WAVES = (6, 10, 12, 14, 21)  # re-fire the HAM clock gate in the cold steady
                          # phase; under the v4 single-table engine balance
                          # warm stretches survive ~50us in steady state
FORCE_BATCH = False       # force same-stationary matmul batches via
                          # tile_wait_until so LDWEIGHTS pipelines (sim-only
                          # timestamps; the scheduler keeps emission order
                          # within a batch released at one instant)
WARMUP_MM = 50            # dummy FD512 matmuls at t=0: fire the PE HAM clock
                          # gate (K=8/8, 2.4GHz) during the init-DMA window so
                          # the GRU phase starts warm instead of 1.2GHz-cold;
                          # sized to stream until ~22us when wave-1 X has landed
TAIL_SPLIT = False        # waves with <=2 active groups: split each group
                          # into two independent FD-256 half-chains. Measured
                          # NEUTRAL (540/548 vs v4 533-548): the tail runs
                          # warmer and PE-busier but per-op overheads and sem
                          # hops eat the chain-latency gain. Kept for re-test.
U_GP_MAX_STEPS = 21       # groups with steps <= this run u = h - n on GpSimd
V_GP_PARITY = False       # v = z*u runs on GpSimd on even steps
X_FP8 = False             # x-side fp8 DoubleRow measured SLOWER on HW (562ns
                          # vs 413ns bf16 at FD512) — keep bf16
HEAD_INLINE = False       # inline heads measured slower (engine in-order
                          # queues stall the GRU pipeline); keep at end

TRACE = False             # set by test harness for profiling
LAST_RESULT = None        # BassKernelResults stash for the harness

_f32 = mybir.dt.float32
_bf16 = mybir.dt.bfloat16
_fp8 = mybir.dt.float8e4
_np_fp8 = mybir.dt.np(_fp8)


def _sigmoid(x):
    return 1.0 / (1.0 + np.exp(-x))


def _build_nc(group_steps, dt):
    """Build the SPMD kernel. group_steps[g] = GRU steps (beyond the constant
    step 0) for group g — identical on every core."""
    AF = mybir.ActivationFunctionType
    OP = mybir.AluOpType
    nc = bacc.Bacc("TRN2", target_bir_lowering=False, debug=False)

    xdt = _fp8 if X_FP8 else dt
    if X_FP8:
        X = nc.dram_tensor("X", [T - 1, KR // 2, G * 2 * SW], xdt, kind="ExternalInput")
        WXR = nc.dram_tensor("WXR", [KR // 2, 2, 128], xdt, kind="ExternalInput")
        WXZ = nc.dram_tensor("WXZ", [KR // 2, 2, 128], xdt, kind="ExternalInput")
        WXN = nc.dram_tensor("WXN", [KR // 2, 2, 128], xdt, kind="ExternalInput")
    else:
        X = nc.dram_tensor("X", [T - 1, KR, G * SW], dt, kind="ExternalInput")
        WXR = nc.dram_tensor("WXR", [KR, 128], dt, kind="ExternalInput")
        WXZ = nc.dram_tensor("WXZ", [KR, 128], dt, kind="ExternalInput")
        WXN = nc.dram_tensor("WXN", [KR, 128], dt, kind="ExternalInput")
    WHR = nc.dram_tensor("WHR", [128, 128], dt, kind="ExternalInput")
    WHZ = nc.dram_tensor("WHZ", [128, 128], dt, kind="ExternalInput")
    WHN = nc.dram_tensor("WHN", [128, 128], dt, kind="ExternalInput")
    IDN = nc.dram_tensor("IDN", [128, 128], dt, kind="ExternalInput")
    W1A = nc.dram_tensor("W1A", [128, FC], dt, kind="ExternalInput")
    W1B = nc.dram_tensor("W1B", [128, FC], dt, kind="ExternalInput")
    W2T = nc.dram_tensor("W2T", [FC, OUT], dt, kind="ExternalInput")
    BHN = nc.dram_tensor("BHN", [128, 1], _f32, kind="ExternalInput")
    B1 = nc.dram_tensor("B1", [128, FC // 128], _f32, kind="ExternalInput")
    B2H = nc.dram_tensor("B2H", [128, OTILES], _f32, kind="ExternalInput")
    H1F = nc.dram_tensor("H1F", [128, 1], _f32, kind="ExternalInput")
    OT = nc.dram_tensor("OT", [OUT, BC], dt, kind="ExternalOutput")

    with tile.TileContext(nc) as tc:
        with (
            tc.tile_pool(name="weights", bufs=1) as wp,
            tc.tile_pool(name="xin", bufs=16) as xp,
            tc.tile_pool(name="hstate", bufs=G) as hp,
            tc.tile_pool(name="hfinal", bufs=1) as hf,
            tc.tile_pool(name="gates", bufs=3) as gp,
            tc.tile_pool(name="head", bufs=3) as fp,
            tc.tile_pool(name="frhs", bufs=1) as frp,
            tc.tile_pool(name="outs", bufs=3) as op_,
            tc.tile_pool(name="psA", bufs=2, space="PSUM") as psA,
            tc.tile_pool(name="psB", bufs=2, space="PSUM") as psB,
            tc.tile_pool(name="psC", bufs=2, space="PSUM") as psC,
        ):
            # --- HAM warmup: stream dummy matmuls from t~0 so the PE clock
            # gate un-throttles before the GRU begins; they run during the
            # init-DMA window (first real matmul waits ~20us on DMAs) ---
            if WARMUP_MM:
                wu = wp.tile([128, SW], dt, name="warmup")
                nc.vector.memset(wu[:], 0.0)
                wups = psA.tile([128, 2 * SW], _f32, tag="rz", name="wu_ps")
                for _ in range(WARMUP_MM):
                    nc.tensor.matmul(
                        wups[:, 0:SW], wu[:, 0:128], wu[:], start=True, stop=True
                    )

            # --- load weights/biases once ---
            def wtile(shape, src, dtt=dt):
                t = wp.tile(shape, dtt, name=f"w_{src.name}")
                nc.sync.dma_start(out=t[:], in_=src[:])
                return t

            if X_FP8:
                wxr = wtile([KR // 2, 2, 128], WXR, _fp8)
                wxz = wtile([KR // 2, 2, 128], WXZ, _fp8)
                wxn = wtile([KR // 2, 2, 128], WXN, _fp8)
            else:
                wxr = wtile([KR, 128], WXR)
                wxz = wtile([KR, 128], WXZ)
                wxn = wtile([KR, 128], WXN)
            whr = wtile([128, 128], WHR)
            whz = wtile([128, 128], WHZ)
            whn = wtile([128, 128], WHN)
            idn = wtile([128, 128], IDN)
            bhn = wtile([128, 1], BHN, _f32)
            h1f = wtile([128, 1], H1F, _f32)
            # head weights: allocate now, DMA after the GRU loop is emitted so
            # the wave-1 X tiles win the DMA queue (head needs these ~400us in)
            w1a = wp.tile([128, FC], dt, name="w_W1A")
            w1b = wp.tile([128, FC], dt, name="w_W1B")
            w2a = wp.tile([128, OUT], dt, name="w_W2A")
            w2b = wp.tile([128, OUT], dt, name="w_W2B")
            b1s = wp.tile([128, FC // 128], _f32, name="w_B1")
            b2s = wp.tile([128, OTILES], _f32, name="w_B2H")

            # broadcast h1 (the post-step-0 state, equal for every sample)
            h1bc = wp.tile([128, SW], dt)
            nc.vector.memset(h1bc[:], 0.0)
            nc.vector.tensor_scalar_add(h1bc[:], h1bc[:], h1f[:])

            # --- per-group state ---
            hfs = [
                hf.tile([128, 2 * SW], dt, tag=f"hf{sg}", name=f"hfs{sg}")
                for sg in range(NSG)
            ]

            def hf_slice(g):
                return hfs[g // 2][:, (g % 2) * SW : (g % 2 + 1) * SW]

            grp = []
            for g in range(G):
                ha = hp.tile([128, SW], dt, tag="hpa")
                hb = hp.tile([128, SW], dt, tag="hpb")
                nc.vector.tensor_copy(ha[:], h1bc[:])
                if group_steps[g] == 0:
                    nc.vector.tensor_copy(hf_slice(g), h1bc[:])
                grp.append({"g": g, "cur": ha, "nxt": hb, "xt": None})

            DR = mybir.MatmulPerfMode.DoubleRow if X_FP8 else None

            def emit_p1_x(st, s):
                """X-tile DMA + moving-slice bookkeeping for one group's step."""
                g = st["g"]
                nsteps = group_steps[g]
                c0 = g * SW
                if (s - 1) % XB == 0:
                    nb = min(XB, nsteps - (s - 1))
                    if X_FP8:
                        xt = xp.tile([KR // 2, XB, 2, SW], xdt, tag="xt")
                        nc.sync.dma_start(
                            out=xt[:, 0:nb, :, :],
                            in_=X[
                                s - 1 : s - 1 + nb, :, 2 * c0 : 2 * c0 + 2 * SW
                            ].rearrange("t k b -> k t b"),
                        )
                    else:
                        xt = xp.tile([KR, XB, SW], dt, tag="xt")
                        nc.sync.dma_start(
                            out=xt[:, 0:nb, :],
                            in_=X[s - 1 : s - 1 + nb, :, c0 : c0 + SW].rearrange(
                                "t k b -> k t b"
                            ),
                        )
                    st["xt"] = xt
                if X_FP8:
                    st["xs_"] = st["xt"][:, (s - 1) % XB, :, :]
                else:
                    st["xs_"] = st["xt"][:, (s - 1) % XB, :]

            bstate = {"i": 0}

            def next_bts():
                bstate["i"] += 1
                return 0.030 + bstate["i"] * 0.014

            def emit_pass1(batch, s):
                """rz + hn matmuls batched by stationary weight across the
                group batch (same-lhsT matmuls run back-to-back so LDWEIGHTS
                pipelines into the PE array instead of exposing ~107ns per
                stationary change — keeps the HAM clock gate's busy monitor
                fed), then sigmoid + t1 per group. No op here depends on
                another engine's same-wave round trip."""
                import contextlib
                for st in batch:
                    emit_p1_x(st, s)
                    st["psRZ"] = psA.tile([128, 2 * SW], _f32, tag="rz", name="psRZ")
                    st["psHN"] = psB.tile([128, SW], _f32, tag="hn", name="psHN")
                ctx = (tc.tile_wait_until(next_bts())
                       if FORCE_BATCH and len(batch) > 1
                       else contextlib.nullcontext())
                with ctx:
                    for st in batch:
                        nc.tensor.matmul(
                            st["psRZ"][:, 0:SW], wxr[:], st["xs_"],
                            start=True, stop=False, perf_mode=DR,
                        )
                    for st in batch:
                        nc.tensor.matmul(
                            st["psRZ"][:, 0:SW], whr[:], st["cur"][:],
                            start=False, stop=True,
                        )
                    for st in batch:
                        nc.tensor.matmul(
                            st["psRZ"][:, SW:], wxz[:], st["xs_"],
                            start=True, stop=False, perf_mode=DR,
                        )
                    for st in batch:
                        nc.tensor.matmul(
                            st["psRZ"][:, SW:], whz[:], st["cur"][:],
                            start=False, stop=True,
                        )
                    for st in batch:
                        nc.tensor.matmul(
                            st["psHN"][:], whn[:], st["cur"][:],
                            start=True, stop=True,
                        )
                for st in batch:
                    # sigmoid via tanh: s(x) = 0.5*tanh(x/2) + 0.5, so the
                    # whole kernel lives on the exp+tanh ACT table (direct
                    # AF.Sigmoid lives on a different table and the scheduler
                    # interleaves head ELU exps with the GRU tail, costing 18
                    # ping-pong ACT_TABLE_LOADs of 1283ns each on the chain)
                    rzt = gp.tile([128, 2 * SW], dt, tag="rzt")
                    nc.scalar.activation(
                        rzt[:], st["psRZ"][:], AF.Tanh, scale=0.5
                    )
                    rz = gp.tile([128, 2 * SW], dt, tag="rz")
                    se = (nc.gpsimd
                          if group_steps[st["g"]] <= U_GP_MAX_STEPS
                          else nc.vector)
                    se.tensor_scalar(
                        rz[:], rzt[:], 0.5, 0.5, op0=OP.mult, op1=OP.add
                    )
                    # t1 = r * (hn + b_hn)
                    t1 = gp.tile([128, SW], dt, tag="t1")
                    nc.vector.scalar_tensor_tensor(
                        t1[:], st["psHN"][:], bhn[:], rz[:, 0:SW],
                        op0=OP.add, op1=OP.mult,
                    )
                    st["rz"], st["t1"] = rz, t1

            def emit_pass2(batch, s):
                """xn + t1-identity matmuls (stationary-batched), tanh,
                h-update. By emission time every dependency (t1, rz) was
                produced in pass 1."""
                import contextlib
                for st in batch:
                    st["psXN"] = psC.tile([128, SW], _f32, tag="xn", name="psXN")
                ctx = (tc.tile_wait_until(next_bts())
                       if FORCE_BATCH and len(batch) > 1
                       else contextlib.nullcontext())
                with ctx:
                    for st in batch:
                        nc.tensor.matmul(
                            st["psXN"][:], wxn[:], st["xs_"],
                            start=True, stop=not T2_VIA_PE, perf_mode=DR,
                        )
                    if T2_VIA_PE:
                        for st in batch:
                            nc.tensor.matmul(
                                st["psXN"][:], idn[:], st["t1"][:],
                                start=False, stop=True,
                            )
                for st in batch:
                    g = st["g"]
                    nsteps = group_steps[g]
                    cur = st["cur"]
                    rz, t1 = st["rz"], st["t1"]
                    # n = tanh(xn + b_cn + t1); b_cn rides the X ones-row
                    if T2_VIA_PE:
                        ntin = st["psXN"]
                    else:
                        t2 = gp.tile([128, SW], dt, tag="t2")
                        nc.vector.tensor_add(t2[:], st["psXN"][:], t1[:])
                        ntin = t2
                    nt = gp.tile([128, SW], dt, tag="nt")
                    nc.scalar.activation(nt[:], ntin[:], AF.Tanh)
                    # h' = n + z*(h - n)
                    u = gp.tile([128, SW], dt, tag="u")
                    ew = nc.gpsimd if nsteps <= U_GP_MAX_STEPS else nc.vector
                    ew.tensor_sub(u[:], cur[:], nt[:])
                    v = gp.tile([128, SW], dt, tag="v")
                    vw = nc.gpsimd if (V_GP_PARITY and s % 2 == 0) else nc.vector
                    vw.tensor_mul(v[:], rz[:, SW:], u[:])
                    dst = hf_slice(g) if s == nsteps else st["nxt"][:]
                    nc.vector.tensor_add(dst, nt[:], v[:])
                    if s != nsteps:
                        st["cur"], st["nxt"] = st["nxt"], st["cur"]

            def emit_tail_wave(act, s):
                """Chain-latency-bound waves (<=2 active groups): run each
                group as two independent 256-wide half-chains. psRZ half o
                occupies columns [o*512,(o+1)*512) as its own contiguous
                [r|z] block; psHN/psXN half o uses [o*256,(o+1)*256).
                Two accumulation groups share a PSUM bank, which is safe on
                HW (per-element has_written) but needs skip_group_check."""
                hw_ = SW // 2
                for st in act:
                    emit_p1_x(st, s)
                    st["psRZ"] = psA.tile([128, 2 * SW], _f32, tag="rz", name="psRZt")
                    st["psHN"] = psB.tile([128, SW], _f32, tag="hn", name="psHNt")
                for st in act:
                    for o in (0, 1):
                        base = o * SW
                        ho = slice(o * hw_, (o + 1) * hw_)
                        r_sl = slice(base, base + hw_)
                        z_sl = slice(base + hw_, base + 2 * hw_)
                        xs_ = st["xs_"][:, ho]
                        cur = st["cur"][:, ho]
                        psRZ, psHN = st["psRZ"], st["psHN"]
                        nc.tensor.matmul(psRZ[:, r_sl], wxr[:], xs_,
                                         start=True, stop=False,
                                         skip_group_check=True)
                        nc.tensor.matmul(psRZ[:, r_sl], whr[:], cur,
                                         start=False, stop=True,
                                         skip_group_check=True)
                        nc.tensor.matmul(psRZ[:, z_sl], wxz[:], xs_,
                                         start=True, stop=False,
                                         skip_group_check=True)
                        nc.tensor.matmul(psRZ[:, z_sl], whz[:], cur,
                                         start=False, stop=True,
                                         skip_group_check=True)
                        nc.tensor.matmul(psHN[:, ho], whn[:], cur,
                                         start=True, stop=True,
                                         skip_group_check=True)
                        rzt = gp.tile([128, 2 * SW], dt, tag="rzt")
                        nc.scalar.activation(rzt[:, 0 : 2 * hw_],
                                             psRZ[:, base : base + 2 * hw_],
                                             AF.Tanh, scale=0.5)
                        rz = gp.tile([128, 2 * SW], dt, tag="rz")
                        nc.vector.tensor_scalar(rz[:, 0 : 2 * hw_],
                                                rzt[:, 0 : 2 * hw_],
                                                0.5, 0.5,
                                                op0=OP.mult, op1=OP.add)
                        t1 = gp.tile([128, SW], dt, tag="t1")
                        nc.vector.scalar_tensor_tensor(
                            t1[:, 0:hw_], psHN[:, ho], bhn[:], rz[:, 0:hw_],
                            op0=OP.add, op1=OP.mult)
                        st[f"rz{o}"], st[f"t1{o}"] = rz, t1
                for st in act:
                    st["psXN"] = psC.tile([128, SW], _f32, tag="xn", name="psXNt")
                for st in act:
                    g = st["g"]
                    nsteps = group_steps[g]
                    for o in (0, 1):
                        ho = slice(o * hw_, (o + 1) * hw_)
                        xs_ = st["xs_"][:, ho]
                        cur = st["cur"][:, ho]
                        rz, t1 = st[f"rz{o}"], st[f"t1{o}"]
                        psXN = st["psXN"]
                        nc.tensor.matmul(psXN[:, ho], wxn[:], xs_,
                                         start=True, stop=False,
                                         skip_group_check=True)
                        nc.tensor.matmul(psXN[:, ho], idn[:], t1[:, 0:hw_],
                                         start=False, stop=True,
                                         skip_group_check=True)
                        nt = gp.tile([128, SW], dt, tag="nt")
                        nc.scalar.activation(nt[:, 0:hw_], psXN[:, ho], AF.Tanh)
                        u = gp.tile([128, SW], dt, tag="u")
                        nc.vector.tensor_sub(u[:, 0:hw_], cur, nt[:, 0:hw_])
                        v = gp.tile([128, SW], dt, tag="v")
                        nc.vector.tensor_mul(v[:, 0:hw_], rz[:, hw_ : 2 * hw_],
                                             u[:, 0:hw_])
                        dst = (hf_slice(g)[:, ho] if s == nsteps
                               else st["nxt"][:, ho])
                        nc.vector.tensor_add(dst, nt[:, 0:hw_], v[:, 0:hw_])
                    if s != nsteps:
                        st["cur"], st["nxt"] = st["nxt"], st["cur"]

            f2rhs = {}

            def emit_head_elu(sg):
                """W1 + ELU for super-group sg, on the psB/psC banks (idle
                once the GRU drains) so the last ELUs overlap the first W2
                tiles, which own the psA ring."""
                for si, w1 in enumerate((w1a, w1b)):
                    for j in range(FC // 128):
                        jc = slice(j * 128, (j + 1) * 128)
                        fr = frp.tile([128, 2 * SW], dt, tag=f"fr{sg}{si}{j}",
                                      name=f"fr{sg}{si}{j}")
                        for ci, (pool, ptag) in enumerate(
                            ((psB, "hn"), (psC, "xn"))
                        ):
                            cs = slice(ci * SW, (ci + 1) * SW)
                            pf = pool.tile([128, SW], _f32, tag=ptag, name="pf")
                            nc.tensor.matmul(
                                pf[:], w1[:, jc], hfs[sg][:, cs],
                                start=True, stop=True,
                            )
                            # elu(y)+1 = exp(min(y,0)) + (y - min(y,0)), y=x+b1
                            m = fp.tile([128, SW], dt, tag="m")
                            nc.vector.tensor_scalar(
                                m[:], pf[:], b1s[:, j : j + 1], 0.0,
                                op0=OP.add, op1=OP.min,
                            )
                            e = fp.tile([128, SW], dt, tag="e")
                            nc.scalar.activation(e[:], m[:], AF.Exp)
                            pp = fp.tile([128, SW], dt, tag="pp")
                            nc.vector.scalar_tensor_tensor(
                                pp[:], pf[:], b1s[:, j : j + 1], m[:],
                                op0=OP.add, op1=OP.subtract,
                            )
                            nc.vector.tensor_add(fr[:, cs], pp[:], e[:])
                        f2rhs[(sg, si, j)] = fr

            def emit_head_w2(sg):
                for si in range(2):
                    for mt in range(OTILES):
                        mw = min(128, OUT - mt * 128)
                        po = psA.tile([128, 2 * SW], _f32, tag="rz", name="po")
                        mc = slice(mt * 128, mt * 128 + mw)
                        for cs in (slice(0, SW), slice(SW, 2 * SW)):
                            nc.tensor.matmul(
                                po[0:mw, cs], w2a[:, mc], f2rhs[(sg, si, 0)][:, cs],
                                start=True, stop=False,
                            )
                            nc.tensor.matmul(
                                po[0:mw, cs], w2b[:, mc], f2rhs[(sg, si, 1)][:, cs],
                                start=False, stop=True,
                            )
                        oth = op_.tile([128, 2 * SW], dt, tag="ot")
                        # store tanh((x+b2)/2); host applies (t+1)/2
                        nc.scalar.activation(
                            oth[0:mw, :], po[0:mw, :], AF.Tanh,
                            bias=b2s[0:mw, mt : mt + 1], scale=0.5,
                        )
                        cb = sg * 4 * SW + si * 2 * SW
                        nc.sync.dma_start(
                            out=OT[mt * 128 : mt * 128 + mw, cb : cb + 2 * SW],
                            in_=oth[0:mw, :],
                        )

            # --- GRU loop: all groups interleaved, two passes per wave so
            # no in-order engine queue blocks on a same-wave round trip;
            # longest-remaining group first so its serial chain is served
            # earliest each wave ---
            def rewarm(n, tag_name):
                """Dense same-stationary dummy burst: covers >=1 full 3413ns
                HAM window with 100%-busy PE streaming so Activity_SHORT
                un-throttles the clock gate (K=8/8) mid-kernel."""
                ps = psA.tile([128, 2 * SW], _f32, tag="rz", name=tag_name)
                for _ in range(n):
                    nc.tensor.matmul(
                        ps[:, 0:SW], wu[:, 0:128], wu[:], start=True, stop=True
                    )

            maxs = max(group_steps)
            for s in range(1, maxs + 1):
                act = [st for st in reversed(grp) if s <= group_steps[st["g"]]]
                # pairs of groups share one PSUM ring generation (bufs=2)
                if TAIL_SPLIT and len(act) <= 2:
                    emit_tail_wave(act, s)
                else:
                    pairs = [act[i : i + 2] for i in range(0, len(act), 2)]
                    for batch in pairs:
                        emit_pass1(batch, s)
                    for batch in pairs:
                        emit_pass2(batch, s)
                if s in REWARM_WAVES:
                    # dependency stalls re-throttle the PE clock (the HAM
                    # Activity_MID monitor); periodically re-fire it — each
                    # burst buys ~15-25us of 2x-clock execution for ~4us cost
                    rewarm(16, f"rewarm{s}")
            # head weights load behind the X stream (see note above)
            nc.sync.dma_start(out=w1a[:], in_=W1A[:])
            nc.sync.dma_start(out=w1b[:], in_=W1B[:])
            nc.sync.dma_start(out=w2a[:], in_=W2T[0:128, :])
            nc.sync.dma_start(out=w2b[:], in_=W2T[128:256, :])
            nc.sync.dma_start(out=b1s[:], in_=B1[:])
            nc.sync.dma_start(out=b2s[:], in_=B2H[:])
            # two global head phases: all ELUs (Exp table), then all W2
            # sigmoids — exactly two activation-table swaps
            for sg in range(NSG):
                emit_head_elu(sg)
            for sg in range(NSG):
                emit_head_w2(sg)

    nc.compile()
    return nc


def kernel(message, lengths, init_emb, W_emb, b_emb, W_ih, W_hh, b_ih, b_hh,
           W1, b1, W2, b2):
    global LAST_RESULT
    message = np.asarray(message, dtype=np.float32)
    lengths = np.asarray(lengths).astype(np.int64)
    f8 = np.float64
    np_dt = ml_dtypes.bfloat16
    dt = _bf16

    # --- fold embedding into input weights;  step 0 is a constant ---
    W_c = W_ih.astype(f8) @ W_emb.astype(f8)                # [3H, V]
    b_c = W_ih.astype(f8) @ b_emb.astype(f8) + b_ih         # [3H]
    gx0 = W_ih.astype(f8) @ init_emb.astype(f8) + b_ih
    gh0 = b_hh.astype(f8)
    r0 = _sigmoid(gx0[:H] + gh0[:H])
    z0 = _sigmoid(gx0[H : 2 * H] + gh0[H : 2 * H])
    n0 = np.tanh(gx0[2 * H :] + r0 * gh0[2 * H :])
    h1 = (1.0 - z0) * n0                                    # h after step 0

    # --- length-sort, stratify across cores ---
    perm = np.argsort(lengths, kind="stable")
    lsort = lengths[perm]
    # group g (on every core) covers global sorted ranks [g*8192, (g+1)*8192)
    group_steps = [int(lsort[min((g + 1) * 2 * SW * NCORES, B) - 1]) - 1
                   for g in range(G)]

    # --- shared weight tensors ---
    Whh = W_hh.astype(f8)
    b_rz = (b_c[: 2 * H] + b_hh[: 2 * H]).astype(f8)
    b_cn = b_c[2 * H :].astype(f8)
    b_hn = b_hh[2 * H :].astype(f8)

    np_xdt = _np_fp8 if X_FP8 else np_dt

    def wx_block(gate, freeze, bias_row):
        w = np.zeros((KR, 128), f8)
        blk = W_c[gate * H : (gate + 1) * H, :].T            # [V, H]
        for sblk in range(2):
            r0_, c0_ = 32 * sblk, 64 * sblk
            w[r0_ : r0_ + V, c0_ : c0_ + H] = blk
            w[r0_ + V, c0_ : c0_ + H] = freeze
            w[r0_ + V + 1, c0_ : c0_ + H] = bias_row
        if X_FP8:
            # DoubleRow layout: [32, 2, 128] with plane p = rows 32p..32p+31
            return np.ascontiguousarray(w.reshape(2, 32, 128).transpose(1, 0, 2)).astype(np_xdt)
        return w.astype(np_dt)

    wxr_d = wx_block(0, 0.0, b_rz[:H])
    wxz_d = wx_block(1, FREEZE, b_rz[H:])
    wxn_d = wx_block(2, 0.0, b_cn)

    def wh_block(gate):
        w = np.zeros((128, 128), f8)
        blk = Whh[gate * H : (gate + 1) * H, :].T            # [H(k), H(f)]
        w[0:H, 0:H] = blk
        w[H:128, H:128] = blk
        return w.astype(np_dt)

    whr_d = wh_block(0)
    whz_d = wh_block(1)
    whn_d = wh_block(2)
    idn_d = np.eye(128, dtype=np_dt)

    w1t = np.ascontiguousarray(W1.T).astype(f8)             # [H, FC]
    w1a_d = np.zeros((128, FC), f8)
    w1a_d[0:H] = w1t
    w1b_d = np.zeros((128, FC), f8)
    w1b_d[H:128] = w1t
    w1a_d = w1a_d.astype(np_dt)
    w1b_d = w1b_d.astype(np_dt)
    w2_d = np.ascontiguousarray(W2.T).astype(np_dt)         # [FC, OUT]

    bhn_d = np.ascontiguousarray(np.tile(b_hn, 2).reshape(128, 1)).astype(np.float32)
    b1_d = np.ascontiguousarray(np.asarray(b1, np.float32).reshape(FC // 128, 128).T)
    b2f = np.asarray(b2, f8) - W2.astype(f8).sum(axis=1)
    b2p = np.zeros(OTILES * 128, np.float32)
    b2p[:OUT] = b2f * 0.5
    b2_d = np.ascontiguousarray(b2p.reshape(OTILES, 128).T)
    h1_d = np.ascontiguousarray(np.tile(h1, 2).reshape(128, 1)).astype(np.float32)

    # --- per-core inputs ---
    trange = np.arange(T - 1)
    # column j of group g: stratum A sample = local rank g*1024 + j,
    # stratum B sample = g*1024 + 512 + j
    cols = np.arange(G * SW)
    a_rank = (cols // SW) * 2 * SW + (cols % SW)
    b_rank = a_rank + SW
    in_maps = []
    core_idx = []
    for c in range(NCORES):
        ic = perm[c::NCORES]
        core_idx.append(ic)
        mc = message[ic][:, : T - 1, :]                     # [BC, 29, 21]
        lc = lengths[ic]
        Xc = np.zeros((T - 1, KR, G * SW), dtype=np_xdt)
        Xc[:, 0:V, :] = mc[a_rank].transpose(1, 2, 0).astype(np_xdt)
        Xc[:, V, :] = (lc[a_rank][None, :] <= trange[:, None] + 1).astype(np_xdt)
        Xc[:, V + 1, :] = 1.0
        Xc[:, 32 : 32 + V, :] = mc[b_rank].transpose(1, 2, 0).astype(np_xdt)
        Xc[:, 32 + V, :] = (lc[b_rank][None, :] <= trange[:, None] + 1).astype(np_xdt)
        Xc[:, 32 + V + 1, :] = 1.0
        if X_FP8:
            # DoubleRow layout [T-1, 32, G*2*SW]: per group, plane p (= x rows
            # 32p..32p+31) occupies the p-th SW-column block
            Xc = np.ascontiguousarray(
                Xc.reshape(T - 1, 2, 32, G, SW)
                .transpose(0, 2, 3, 1, 4)
                .reshape(T - 1, 32, G * 2 * SW)
            )
        in_maps.append({
            "X": Xc, "WXR": wxr_d, "WXZ": wxz_d, "WXN": wxn_d,
            "WHR": whr_d, "WHZ": whz_d, "WHN": whn_d, "IDN": idn_d,
            "W1A": w1a_d, "W1B": w1b_d, "W2T": w2_d,
            "BHN": bhn_d, "B1": b1_d, "B2H": b2_d, "H1F": h1_d,
        })

    nc = _build_nc(group_steps, dt)
    res = run_bass_kernel_spmd(nc, in_maps, core_ids=list(range(NCORES)), trace=TRACE)
    LAST_RESULT = res

    # OT column k -> local sorted rank
    k = np.arange(BC)
    sg_ = k // (4 * SW)
    si_ = (k % (4 * SW)) // (2 * SW)
    c_ = k % (2 * SW)
    g_ = 2 * sg_ + (c_ >= SW)
    L = g_ * 2 * SW + si_ * SW + (c_ % SW)

    out = np.empty((B, OUT), np.float32)
    for c in range(NCORES):
        out[core_idx[c][L]] = (res.results[c]["OT"].astype(np.float32).T + 1.0) * 0.5
    return out



# revision 37
# speedup vs baseline: 1.2275x; 1.2275x over previous
"""Trainium2 Bass kernel for nn_Decoder (GRU decoder over padded sequences).

Computation (per sample):
  emb = message[:, :T-1] @ W_emb.T + b_emb            (folded into W_c on host)
  xs  = [init_emb, emb]                                (step 0 folded into h1 const)
  GRU over T steps, gather h at lengths-1              (freeze trick: z := 1 past len)
  out = sigmoid(elu(h @ W1.T + b1) @ W2.T + b2)

Sharding: batch data-parallel over 8 cores, host-side length-sort (stratified
round-robin across cores) so each group only runs to its max length.

v2 layout: two 512-sample strata are packed on the partition axis (stratum A
on partitions 0..63, B on 64..127) via block-diagonal weight matrices, so
every elementwise op runs with all 128 partitions at free-dim 512 (DVE/ACT
cost scales with free size only). Sigmoids are computed as
0.5*tanh(x/2)+0.5 so the whole kernel lives in one ACT table (Tanh+Exp, no
table swaps) and r+z share a single [128,1024] tanh. Gate biases ride a
constant ones-row in the streamed X tile; b_hn is applied by the DVE
scalar_tensor_tensor that forms r*(hn+b_hn). The xn + r*hn sum is
accumulated in PSUM by an identity matmul on the TensorEngine.

v4: sigmoids via tanh: rz gates = 0.5*tanh(x/2)+0.5 (ACT scale=0.5 pre-affine,
+1 tensor_scalar on gpsimd/DVE), W2 output stored as tanh((x+b2)/2) with the
(t+1)/2 affine done on host in f32 (free + more accurate). Direct AF.Sigmoid
lives on a different ACT table than AF.Exp (ELU), and the scheduler was
interleaving head ELU exps into the GRU tail: 18 ping-pong ACT_TABLE_LOADs
(1283ns each) sat on the ACT queue right where the tail chain is
latency-bound. Single-table: 1 load total, -26us + better accuracy (6.8e-4).

v3 HAM notes (measured on HW): the PE clock gate (HAM) keeps the array at
1.2GHz cold unless a 4096-cycle window is ~fully busy with array STREAMING
(LDWEIGHTS time does not count). Steady GRU = 80% array-busy (each of the 7
stationary changes/group-step exposes ~107ns LDWEIGHTS) so it never warms on
its own; the 50-matmul same-stationary warmup burst at t=0 streams 100%-dense
during the ~20us init-DMA window and fires K=8/8 for the first ~2 waves.
v4: single ACT table (see below) changed the balance — 3 mid-GRU re-warm
bursts at waves 6/10/14 now NET HELP (-28us; warm stretches survive 30-75us).
Denser bursts (10 of them) measured WORSE (+20us of redundant streaming).
Still not helping: forced same-stationary pair-batching via tile_wait_until
(raises density to 89% and SHORT does re-fire, but at 2.4GHz array-busy drops
to 82% and Activity_MID immediately re-throttles; the reorder also lengthens
the pair serial chain, hurting the tail), T2_VIA_PE=False (+100us DVE).
Head weights are DMA'd after the GRU loop so wave-1 X wins the DMA queue.
Beware: the chip drops to a ~2.0GHz PLL state (P0) after sustained load —
run-to-run exec varies 590-730ns; compare only like-state runs (cold-matmul
duration 605-612 = fast state, ~730 = P0).
"""

import sys

sys.path.insert(0, "/opt/trn_rl_repo")

import numpy as np
import ml_dtypes

import concourse.bacc as bacc
import concourse.mybir as mybir
import concourse.tile as tile
from concourse.bass_utils import run_bass_kernel_spmd

B, T, V, E, H, FC, OUT = 65536, 30, 21, 32, 64, 256, 784
NCORES = 8
BC = B // NCORES          # 8192 samples per core
SW = 512                  # stratum width (free dim of GRU ops)
G = 8                     # groups per core; group = 2 strata of SW samples
NSG = G // 2              # super-groups for the MLP head (free dim 1024)
FREEZE = 40.0             # z-gate preactivation offset for finished samples
XB = 4                    # GRU steps per streamed X block
OTILES = (OUT + 127) // 128  # 7 output row tiles
KR = 64                   # X tile partition rows (A block 0..31, B block 32..63)

T2_VIA_PE = True          # accumulate t1 into xn PSUM with identity matmul
REWARM_WAVES = (6, 10, 14, 21)  # re-fire the HAM clock gate in the cold steady
                          # phase; under the v4 single-table engine balance
                          # warm stretches survive ~50us in steady state
FORCE_BATCH = False       # force same-stationary matmul batches via
                          # tile_wait_until so LDWEIGHTS pipelines (sim-only
                          # timestamps; the scheduler keeps emission order
                          # within a batch released at one instant)
WARMUP_MM = 50            # dummy FD512 matmuls at t=0: fire the PE HAM clock
                          # gate (K=8/8, 2.4GHz) during the init-DMA window so
                          # the GRU phase starts warm instead of 1.2GHz-cold;
                          # sized to stream until ~22us when wave-1 X has landed
TAIL_SPLIT = False        # waves with <=2 active groups: split each group
                          # into two independent FD-256 half-chains. Measured
                          # NEUTRAL (540/548 vs v4 533-548): the tail runs
                          # warmer and PE-busier but per-op overheads and sem
                          # hops eat the chain-latency gain. Kept for re-test.
U_GP_MAX_STEPS = 21       # groups with steps <= this run u = h - n on GpSimd
V_GP_PARITY = False       # v = z*u runs on GpSimd on even steps
X_FP8 = False             # x-side fp8 DoubleRow measured SLOWER on HW (562ns
                          # vs 413ns bf16 at FD512) — keep bf16
HEAD_INLINE = False       # inline heads measured slower (engine in-order
                          # queues stall the GRU pipeline); keep at end

TRACE = False             # set by test harness for profiling
LAST_RESULT = None        # BassKernelResults stash for the harness

_f32 = mybir.dt.float32
_bf16 = mybir.dt.bfloat16
_fp8 = mybir.dt.float8e4
_np_fp8 = mybir.dt.np(_fp8)


def _sigmoid(x):
    return 1.0 / (1.0 + np.exp(-x))


def _build_nc(group_steps, dt):
    """Build the SPMD kernel. group_steps[g] = GRU steps (beyond the constant
    step 0) for group g — identical on every core."""
    AF = mybir.ActivationFunctionType
    OP = mybir.AluOpType
    nc = bacc.Bacc("TRN2", target_bir_lowering=False, debug=False)

    xdt = _fp8 if X_FP8 else dt
    if X_FP8:
        X = nc.dram_tensor("X", [T - 1, KR // 2, G * 2 * SW], xdt, kind="ExternalInput")
        WXR = nc.dram_tensor("WXR", [KR // 2, 2, 128], xdt, kind="ExternalInput")
        WXZ = nc.dram_tensor("WXZ", [KR // 2, 2, 128], xdt, kind="ExternalInput")
        WXN = nc.dram_tensor("WXN", [KR // 2, 2, 128], xdt, kind="ExternalInput")
    else:
        X = nc.dram_tensor("X", [T - 1, KR, G * SW], dt, kind="ExternalInput")
        WXR = nc.dram_tensor("WXR", [KR, 128], dt, kind="ExternalInput")
        WXZ = nc.dram_tensor("WXZ", [KR, 128], dt, kind="ExternalInput")
        WXN = nc.dram_tensor("WXN", [KR, 128], dt, kind="ExternalInput")
    WHR = nc.dram_tensor("WHR", [128, 128], dt, kind="ExternalInput")
    WHZ = nc.dram_tensor("WHZ", [128, 128], dt, kind="ExternalInput")
    WHN = nc.dram_tensor("WHN", [128, 128], dt, kind="ExternalInput")
    IDN = nc.dram_tensor("IDN", [128, 128], dt, kind="ExternalInput")
    W1A = nc.dram_tensor("W1A", [128, FC], dt, kind="ExternalInput")
    W1B = nc.dram_tensor("W1B", [128, FC], dt, kind="ExternalInput")
    W2T = nc.dram_tensor("W2T", [FC, OUT], dt, kind="ExternalInput")
    BHN = nc.dram_tensor("BHN", [128, 1], _f32, kind="ExternalInput")
    B1 = nc.dram_tensor("B1", [128, FC // 128], _f32, kind="ExternalInput")
    B2H = nc.dram_tensor("B2H", [128, OTILES], _f32, kind="ExternalInput")
    H1F = nc.dram_tensor("H1F", [128, 1], _f32, kind="ExternalInput")
    OT = nc.dram_tensor("OT", [OUT, BC], dt, kind="ExternalOutput")

    with tile.TileContext(nc) as tc:
        with (
            tc.tile_pool(name="weights", bufs=1) as wp,
            tc.tile_pool(name="xin", bufs=16) as xp,
            tc.tile_pool(name="hstate", bufs=G) as hp,
            tc.tile_pool(name="hfinal", bufs=1) as hf,
            tc.tile_pool(name="gates", bufs=3) as gp,
            tc.tile_pool(name="head", bufs=3) as fp,
            tc.tile_pool(name="frhs", bufs=1) as frp,
            tc.tile_pool(name="outs", bufs=3) as op_,
            tc.tile_pool(name="psA", bufs=2, space="PSUM") as psA,
            tc.tile_pool(name="psB", bufs=2, space="PSUM") as psB,
            tc.tile_pool(name="psC", bufs=2, space="PSUM") as psC,
        ):
            # --- HAM warmup: stream dummy matmuls from t~0 so the PE clock
            # gate un-throttles before the GRU begins; they run during the
            # init-DMA window (first real matmul waits ~20us on DMAs) ---
            if WARMUP_MM:
                wu = wp.tile([128, SW], dt, name="warmup")
                nc.vector.memset(wu[:], 0.0)
                wups = psA.tile([128, 2 * SW], _f32, tag="rz", name="wu_ps")
                for _ in range(WARMUP_MM):
                    nc.tensor.matmul(
                        wups[:, 0:SW], wu[:, 0:128], wu[:], start=True, stop=True
                    )

            # --- load weights/biases once ---
            def wtile(shape, src, dtt=dt):
                t = wp.tile(shape, dtt, name=f"w_{src.name}")
                nc.sync.dma_start(out=t[:], in_=src[:])
                return t

            if X_FP8:
                wxr = wtile([KR // 2, 2, 128], WXR, _fp8)
                wxz = wtile([KR // 2, 2, 128], WXZ, _fp8)
                wxn = wtile([KR // 2, 2, 128], WXN, _fp8)
            else:
                wxr = wtile([KR, 128], WXR)
                wxz = wtile([KR, 128], WXZ)
                wxn = wtile([KR, 128], WXN)
            whr = wtile([128, 128], WHR)
            whz = wtile([128, 128], WHZ)
            whn = wtile([128, 128], WHN)
            idn = wtile([128, 128], IDN)
            bhn = wtile([128, 1], BHN, _f32)
            h1f = wtile([128, 1], H1F, _f32)
            # head weights: allocate now, DMA after the GRU loop is emitted so
            # the wave-1 X tiles win the DMA queue (head needs these ~400us in)
            w1a = wp.tile([128, FC], dt, name="w_W1A")
            w1b = wp.tile([128, FC], dt, name="w_W1B")
            w2a = wp.tile([128, OUT], dt, name="w_W2A")
            w2b = wp.tile([128, OUT], dt, name="w_W2B")
            b1s = wp.tile([128, FC // 128], _f32, name="w_B1")
            b2s = wp.tile([128, OTILES], _f32, name="w_B2H")

            # broadcast h1 (the post-step-0 state, equal for every sample)
            h1bc = wp.tile([128, SW], dt)
            nc.vector.memset(h1bc[:], 0.0)
            nc.vector.tensor_scalar_add(h1bc[:], h1bc[:], h1f[:])

            # --- per-group state ---
            hfs = [
                hf.tile([128, 2 * SW], dt, tag=f"hf{sg}", name=f"hfs{sg}")
                for sg in range(NSG)
            ]

            def hf_slice(g):
                return hfs[g // 2][:, (g % 2) * SW : (g % 2 + 1) * SW]

            grp = []
            for g in range(G):
                ha = hp.tile([128, SW], dt, tag="hpa")
                hb = hp.tile([128, SW], dt, tag="hpb")
                nc.vector.tensor_copy(ha[:], h1bc[:])
                if group_steps[g] == 0:
                    nc.vector.tensor_copy(hf_slice(g), h1bc[:])
                grp.append({"g": g, "cur": ha, "nxt": hb, "xt": None})

            DR = mybir.MatmulPerfMode.DoubleRow if X_FP8 else None

            def emit_p1_x(st, s):
                """X-tile DMA + moving-slice bookkeeping for one group's step."""
                g = st["g"]
                nsteps = group_steps[g]
                c0 = g * SW
                if (s - 1) % XB == 0:
                    nb = min(XB, nsteps - (s - 1))
                    if X_FP8:
                        xt = xp.tile([KR // 2, XB, 2, SW], xdt, tag="xt")
                        nc.sync.dma_start(
                            out=xt[:, 0:nb, :, :],
                            in_=X[
                                s - 1 : s - 1 + nb, :, 2 * c0 : 2 * c0 + 2 * SW
                            ].rearrange("t k b -> k t b"),
                        )
                    else:
                        xt = xp.tile([KR, XB, SW], dt, tag="xt")
                        nc.sync.dma_start(
                            out=xt[:, 0:nb, :],
                            in_=X[s - 1 : s - 1 + nb, :, c0 : c0 + SW].rearrange(
                                "t k b -> k t b"
                            ),
                        )
                    st["xt"] = xt
                if X_FP8:
                    st["xs_"] = st["xt"][:, (s - 1) % XB, :, :]
                else:
                    st["xs_"] = st["xt"][:, (s - 1) % XB, :]

            bstate = {"i": 0}

            def next_bts():
                bstate["i"] += 1
                return 0.030 + bstate["i"] * 0.014

            def emit_pass1(batch, s):
                """rz + hn matmuls batched by stationary weight across the
                group batch (same-lhsT matmuls run back-to-back so LDWEIGHTS
                pipelines into the PE array instead of exposing ~107ns per
                stationary change — keeps the HAM clock gate's busy monitor
                fed), then sigmoid + t1 per group. No op here depends on
                another engine's same-wave round trip."""
                import contextlib
                for st in batch:
                    emit_p1_x(st, s)
                    st["psRZ"] = psA.tile([128, 2 * SW], _f32, tag="rz", name="psRZ")
                    st["psHN"] = psB.tile([128, SW], _f32, tag="hn", name="psHN")
                ctx = (tc.tile_wait_until(next_bts())
                       if FORCE_BATCH and len(batch) > 1
                       else contextlib.nullcontext())
                with ctx:
                    for st in batch:
                        nc.tensor.matmul(
                            st["psRZ"][:, 0:SW], wxr[:], st["xs_"],
                            start=True, stop=False, perf_mode=DR,
                        )
                    for st in batch:
                        nc.tensor.matmul(
                            st["psRZ"][:, 0:SW], whr[:], st["cur"][:],
                            start=False, stop=True,
                        )
                    for st in batch:
                        nc.tensor.matmul(
                            st["psRZ"][:, SW:], wxz[:], st["xs_"],
                            start=True, stop=False, perf_mode=DR,
                        )
                    for st in batch:
                        nc.tensor.matmul(
                            st["psRZ"][:, SW:], whz[:], st["cur"][:],
                            start=False, stop=True,
                        )
                    for st in batch:
                        nc.tensor.matmul(
                            st["psHN"][:], whn[:], st["cur"][:],
                            start=True, stop=True,
                        )
                for st in batch:
                    # sigmoid via tanh: s(x) = 0.5*tanh(x/2) + 0.5, so the
                    # whole kernel lives on the exp+tanh ACT table (direct
                    # AF.Sigmoid lives on a different table and the scheduler
                    # interleaves head ELU exps with the GRU tail, costing 18
                    # ping-pong ACT_TABLE_LOADs of 1283ns each on the chain)
                    rzt = gp.tile([128, 2 * SW], dt, tag="rzt")
                    nc.scalar.activation(
                        rzt[:], st["psRZ"][:], AF.Tanh, scale=0.5
                    )
                    rz = gp.tile([128, 2 * SW], dt, tag="rz")
                    se = (nc.gpsimd
                          if group_steps[st["g"]] <= U_GP_MAX_STEPS
                          else nc.vector)
                    se.tensor_scalar(
                        rz[:], rzt[:], 0.5, 0.5, op0=OP.mult, op1=OP.add
                    )
                    # t1 = r * (hn + b_hn)
                    t1 = gp.tile([128, SW], dt, tag="t1")
                    nc.vector.scalar_tensor_tensor(
                        t1[:], st["psHN"][:], bhn[:], rz[:, 0:SW],
                        op0=OP.add, op1=OP.mult,
                    )
                    st["rz"], st["t1"] = rz, t1

            def emit_pass2(batch, s):
                """xn + t1-identity matmuls (stationary-batched), tanh,
                h-update. By emission time every dependency (t1, rz) was
                produced in pass 1."""
                import contextlib
                for st in batch:
                    st["psXN"] = psC.tile([128, SW], _f32, tag="xn", name="psXN")
                ctx = (tc.tile_wait_until(next_bts())
                       if FORCE_BATCH and len(batch) > 1
                       else contextlib.nullcontext())
                with ctx:
                    for st in batch:
                        nc.tensor.matmul(
                            st["psXN"][:], wxn[:], st["xs_"],
                            start=True, stop=not T2_VIA_PE, perf_mode=DR,
                        )
                    if T2_VIA_PE:
                        for st in batch:
                            nc.tensor.matmul(
                                st["psXN"][:], idn[:], st["t1"][:],
                                start=False, stop=True,
                            )
                for st in batch:
                    g = st["g"]
                    nsteps = group_steps[g]
                    cur = st["cur"]
                    rz, t1 = st["rz"], st["t1"]
                    # n = tanh(xn + b_cn + t1); b_cn rides the X ones-row
                    if T2_VIA_PE:
                        ntin = st["psXN"]
                    else:
                        t2 = gp.tile([128, SW], dt, tag="t2")
                        nc.vector.tensor_add(t2[:], st["psXN"][:], t1[:])
                        ntin = t2
                    nt = gp.tile([128, SW], dt, tag="nt")
                    nc.scalar.activation(nt[:], ntin[:], AF.Tanh)
                    # h' = n + z*(h - n)
                    u = gp.tile([128, SW], dt, tag="u")
                    ew = nc.gpsimd if nsteps <= U_GP_MAX_STEPS else nc.vector
                    ew.tensor_sub(u[:], cur[:], nt[:])
                    v = gp.tile([128, SW], dt, tag="v")
                    vw = nc.gpsimd if (V_GP_PARITY and s % 2 == 0) else nc.vector
                    vw.tensor_mul(v[:], rz[:, SW:], u[:])
                    dst = hf_slice(g) if s == nsteps else st["nxt"][:]
                    nc.vector.tensor_add(dst, nt[:], v[:])
                    if s != nsteps:
                        st["cur"], st["nxt"] = st["nxt"], st["cur"]

            def emit_tail_wave(act, s):
                """Chain-latency-bound waves (<=2 active groups): run each
                group as two independent 256-wide half-chains. psRZ half o
                occupies columns [o*512,(o+1)*512) as its own contiguous
                [r|z] block; psHN/psXN half o uses [o*256,(o+1)*256).
                Two accumulation groups share a PSUM bank, which is safe on
                HW (per-element has_written) but needs skip_group_check."""
                hw_ = SW // 2
                for st in act:
                    emit_p1_x(st, s)
                    st["psRZ"] = psA.tile([128, 2 * SW], _f32, tag="rz", name="psRZt")
                    st["psHN"] = psB.tile([128, SW], _f32, tag="hn", name="psHNt")
                for st in act:
                    for o in (0, 1):
                        base = o * SW
                        ho = slice(o * hw_, (o + 1) * hw_)
                        r_sl = slice(base, base + hw_)
                        z_sl = slice(base + hw_, base + 2 * hw_)
                        xs_ = st["xs_"][:, ho]
                        cur = st["cur"][:, ho]
                        psRZ, psHN = st["psRZ"], st["psHN"]
                        nc.tensor.matmul(psRZ[:, r_sl], wxr[:], xs_,
                                         start=True, stop=False,
                                         skip_group_check=True)
                        nc.tensor.matmul(psRZ[:, r_sl], whr[:], cur,
                                         start=False, stop=True,
                                         skip_group_check=True)
                        nc.tensor.matmul(psRZ[:, z_sl], wxz[:], xs_,
                                         start=True, stop=False,
                                         skip_group_check=True)
                        nc.tensor.matmul(psRZ[:, z_sl], whz[:], cur,
                                         start=False, stop=True,
                                         skip_group_check=True)
                        nc.tensor.matmul(psHN[:, ho], whn[:], cur,
                                         start=True, stop=True,
                                         skip_group_check=True)
                        rzt = gp.tile([128, 2 * SW], dt, tag="rzt")
                        nc.scalar.activation(rzt[:, 0 : 2 * hw_],
                                             psRZ[:, base : base + 2 * hw_],
                                             AF.Tanh, scale=0.5)
                        rz = gp.tile([128, 2 * SW], dt, tag="rz")
                        nc.vector.tensor_scalar(rz[:, 0 : 2 * hw_],
                                                rzt[:, 0 : 2 * hw_],
                                                0.5, 0.5,
                                                op0=OP.mult, op1=OP.add)
                        t1 = gp.tile([128, SW], dt, tag="t1")
                        nc.vector.scalar_tensor_tensor(
                            t1[:, 0:hw_], psHN[:, ho], bhn[:], rz[:, 0:hw_],
                            op0=OP.add, op1=OP.mult)
                        st[f"rz{o}"], st[f"t1{o}"] = rz, t1
                for st in act:
                    st["psXN"] = psC.tile([128, SW], _f32, tag="xn", name="psXNt")
                for st in act:
                    g = st["g"]
                    nsteps = group_steps[g]
                    for o in (0, 1):
                        ho = slice(o * hw_, (o + 1) * hw_)
                        xs_ = st["xs_"][:, ho]
                        cur = st["cur"][:, ho]
                        rz, t1 = st[f"rz{o}"], st[f"t1{o}"]
                        psXN = st["psXN"]
                        nc.tensor.matmul(psXN[:, ho], wxn[:], xs_,
                                         start=True, stop=False,
                                         skip_group_check=True)
                        nc.tensor.matmul(psXN[:, ho], idn[:], t1[:, 0:hw_],
                                         start=False, stop=True,
                                         skip_group_check=True)
                        nt = gp.tile([128, SW], dt, tag="nt")
                        nc.scalar.activation(nt[:, 0:hw_], psXN[:, ho], AF.Tanh)
                        u = gp.tile([128, SW], dt, tag="u")
                        nc.vector.tensor_sub(u[:, 0:hw_], cur, nt[:, 0:hw_])
                        v = gp.tile([128, SW], dt, tag="v")
                        nc.vector.tensor_mul(v[:, 0:hw_], rz[:, hw_ : 2 * hw_],
                                             u[:, 0:hw_])
                        dst = (hf_slice(g)[:, ho] if s == nsteps
                               else st["nxt"][:, ho])
                        nc.vector.tensor_add(dst, nt[:, 0:hw_], v[:, 0:hw_])
                    if s != nsteps:
                        st["cur"], st["nxt"] = st["nxt"], st["cur"]

            f2rhs = {}

            def emit_head_elu(sg):
                """W1 + ELU for super-group sg, on the psB/psC banks (idle
                once the GRU drains) so the last ELUs overlap the first W2
                tiles, which own the psA ring."""
                for si, w1 in enumerate((w1a, w1b)):
                    for j in range(FC // 128):
                        jc = slice(j * 128, (j + 1) * 128)
                        fr = frp.tile([128, 2 * SW], dt, tag=f"fr{sg}{si}{j}",
                                      name=f"fr{sg}{si}{j}")
                        for ci, (pool, ptag) in enumerate(
                            ((psB, "hn"), (psC, "xn"))
                        ):
                            cs = slice(ci * SW, (ci + 1) * SW)
                            pf = pool.tile([128, SW], _f32, tag=ptag, name="pf")
                            nc.tensor.matmul(
                                pf[:], w1[:, jc], hfs[sg][:, cs],
                                start=True, stop=True,
                            )
                            # elu(y)+1 = exp(min(y,0)) + (y - min(y,0)), y=x+b1
                            m = fp.tile([128, SW], dt, tag="m")
                            nc.vector.tensor_scalar(
                                m[:], pf[:], b1s[:, j : j + 1], 0.0,
                                op0=OP.add, op1=OP.min,
                            )
                            e = fp.tile([128, SW], dt, tag="e")
                            nc.scalar.activation(e[:], m[:], AF.Exp)
                            pp = fp.tile([128, SW], dt, tag="pp")
                            nc.vector.scalar_tensor_tensor(
                                pp[:], pf[:], b1s[:, j : j + 1], m[:],
                                op0=OP.add, op1=OP.subtract,
                            )
                            nc.vector.tensor_add(fr[:, cs], pp[:], e[:])
                        f2rhs[(sg, si, j)] = fr

            def emit_head_w2(sg):
                for si in range(2):
                    for mt in range(OTILES):
                        mw = min(128, OUT - mt * 128)
                        po = psA.tile([128, 2 * SW], _f32, tag="rz", name="po")
                        mc = slice(mt * 128, mt * 128 + mw)
                        for cs in (slice(0, SW), slice(SW, 2 * SW)):
                            nc.tensor.matmul(
                                po[0:mw, cs], w2a[:, mc], f2rhs[(sg, si, 0)][:, cs],
                                start=True, stop=False,
                            )
                            nc.tensor.matmul(
                                po[0:mw, cs], w2b[:, mc], f2rhs[(sg, si, 1)][:, cs],
                                start=False, stop=True,
                            )
                        oth = op_.tile([128, 2 * SW], dt, tag="ot")
                        # store tanh((x+b2)/2); host applies (t+1)/2
                        nc.scalar.activation(
                            oth[0:mw, :], po[0:mw, :], AF.Tanh,
                            bias=b2s[0:mw, mt : mt + 1], scale=0.5,
                        )
                        cb = sg * 4 * SW + si * 2 * SW
                        nc.sync.dma_start(
                            out=OT[mt * 128 : mt * 128 + mw, cb : cb + 2 * SW],
                            in_=oth[0:mw, :],
                        )

            # --- GRU loop: all groups interleaved, two passes per wave so
            # no in-order engine queue blocks on a same-wave round trip;
            # longest-remaining group first so its serial chain is served
            # earliest each wave ---
            def rewarm(n, tag_name):
                """Dense same-stationary dummy burst: covers >=1 full 3413ns
                HAM window with 100%-busy PE streaming so Activity_SHORT
                un-throttles the clock gate (K=8/8) mid-kernel."""
                ps = psA.tile([128, 2 * SW], _f32, tag="rz", name=tag_name)
                for _ in range(n):
                    nc.tensor.matmul(
                        ps[:, 0:SW], wu[:, 0:128], wu[:], start=True, stop=True
                    )

            maxs = max(group_steps)
            for s in range(1, maxs + 1):
                act = [st for st in reversed(grp) if s <= group_steps[st["g"]]]
                # pairs of groups share one PSUM ring generation (bufs=2)
                if TAIL_SPLIT and len(act) <= 2:
                    emit_tail_wave(act, s)
                else:
                    pairs = [act[i : i + 2] for i in range(0, len(act), 2)]
                    for batch in pairs:
                        emit_pass1(batch, s)
                    for batch in pairs:
                        emit_pass2(batch, s)
                if s in REWARM_WAVES:
                    # dependency stalls re-throttle the PE clock (the HAM
                    # Activity_MID monitor); periodically re-fire it — each
                    # burst buys ~15-25us of 2x-clock execution for ~4us cost
                    rewarm(16, f"rewarm{s}")
            # head weights load behind the X stream (see note above)
            nc.sync.dma_start(out=w1a[:], in_=W1A[:])
            nc.sync.dma_start(out=w1b[:], in_=W1B[:])
            nc.sync.dma_start(out=w2a[:], in_=W2T[0:128, :])
            nc.sync.dma_start(out=w2b[:], in_=W2T[128:256, :])
            nc.sync.dma_start(out=b1s[:], in_=B1[:])
            nc.sync.dma_start(out=b2s[:], in_=B2H[:])
            # two global head phases: all ELUs (Exp table), then all W2
            # sigmoids — exactly two activation-table swaps
            for sg in range(NSG):
                emit_head_elu(sg)
            for sg in range(NSG):
                emit_head_w2(sg)

    nc.compile()
    return nc


def kernel(message, lengths, init_emb, W_emb, b_emb, W_ih, W_hh, b_ih, b_hh,
           W1, b1, W2, b2):
    global LAST_RESULT
    message = np.asarray(message, dtype=np.float32)
    lengths = np.asarray(lengths).astype(np.int64)
    f8 = np.float64
    np_dt = ml_dtypes.bfloat16
    dt = _bf16

    # --- fold embedding into input weights;  step 0 is a constant ---
    W_c = W_ih.astype(f8) @ W_emb.astype(f8)                # [3H, V]
    b_c = W_ih.astype(f8) @ b_emb.astype(f8) + b_ih         # [3H]
    gx0 = W_ih.astype(f8) @ init_emb.astype(f8) + b_ih
    gh0 = b_hh.astype(f8)
    r0 = _sigmoid(gx0[:H] + gh0[:H])
    z0 = _sigmoid(gx0[H : 2 * H] + gh0[H : 2 * H])
    n0 = np.tanh(gx0[2 * H :] + r0 * gh0[2 * H :])
    h1 = (1.0 - z0) * n0                                    # h after step 0

    # --- length-sort, stratify across cores ---
    perm = np.argsort(lengths, kind="stable")
    lsort = lengths[perm]
    # group g (on every core) covers global sorted ranks [g*8192, (g+1)*8192)
    group_steps = [int(lsort[min((g + 1) * 2 * SW * NCORES, B) - 1]) - 1
                   for g in range(G)]

    # --- shared weight tensors ---
    Whh = W_hh.astype(f8)
    b_rz = (b_c[: 2 * H] + b_hh[: 2 * H]).astype(f8)
    b_cn = b_c[2 * H :].astype(f8)
    b_hn = b_hh[2 * H :].astype(f8)

    np_xdt = _np_fp8 if X_FP8 else np_dt

    def wx_block(gate, freeze, bias_row):
        w = np.zeros((KR, 128), f8)
        blk = W_c[gate * H : (gate + 1) * H, :].T            # [V, H]
        for sblk in range(2):
            r0_, c0_ = 32 * sblk, 64 * sblk
            w[r0_ : r0_ + V, c0_ : c0_ + H] = blk
            w[r0_ + V, c0_ : c0_ + H] = freeze
            w[r0_ + V + 1, c0_ : c0_ + H] = bias_row
        if X_FP8:
            # DoubleRow layout: [32, 2, 128] with plane p = rows 32p..32p+31
            return np.ascontiguousarray(w.reshape(2, 32, 128).transpose(1, 0, 2)).astype(np_xdt)
        return w.astype(np_dt)

    wxr_d = wx_block(0, 0.0, b_rz[:H])
    wxz_d = wx_block(1, FREEZE, b_rz[H:])
    wxn_d = wx_block(2, 0.0, b_cn)

    def wh_block(gate):
        w = np.zeros((128, 128), f8)
        blk = Whh[gate * H : (gate + 1) * H, :].T            # [H(k), H(f)]
        w[0:H, 0:H] = blk
        w[H:128, H:128] = blk
        return w.astype(np_dt)

    whr_d = wh_block(0)
    whz_d = wh_block(1)
    whn_d = wh_block(2)
    idn_d = np.eye(128, dtype=np_dt)

    w1t = np.ascontiguousarray(W1.T).astype(f8)             # [H, FC]
    w1a_d = np.zeros((128, FC), f8)
    w1a_d[0:H] = w1t
    w1b_d = np.zeros((128, FC), f8)
    w1b_d[H:128] = w1t
    w1a_d = w1a_d.astype(np_dt)
    w1b_d = w1b_d.astype(np_dt)
    w2_d = np.ascontiguousarray(W2.T).astype(np_dt)         # [FC, OUT]

    bhn_d = np.ascontiguousarray(np.tile(b_hn, 2).reshape(128, 1)).astype(np.float32)
    b1_d = np.ascontiguousarray(np.asarray(b1, np.float32).reshape(FC // 128, 128).T)
    b2f = np.asarray(b2, f8) - W2.astype(f8).sum(axis=1)
    b2p = np.zeros(OTILES * 128, np.float32)
    b2p[:OUT] = b2f * 0.5
    b2_d = np.ascontiguousarray(b2p.reshape(OTILES, 128).T)
    h1_d = np.ascontiguousarray(np.tile(h1, 2).reshape(128, 1)).astype(np.float32)

    # --- per-core inputs ---
    trange = np.arange(T - 1)
    # column j of group g: stratum A sample = local rank g*1024 + j,
    # stratum B sample = g*1024 + 512 + j
    cols = np.arange(G * SW)
    a_rank = (cols // SW) * 2 * SW + (cols % SW)
    b_rank = a_rank + SW
    in_maps = []
    core_idx = []
    for c in range(NCORES):
        ic = perm[c::NCORES]
        core_idx.append(ic)
        mc = message[ic][:, : T - 1, :]                     # [BC, 29, 21]
        lc = lengths[ic]
        Xc = np.zeros((T - 1, KR, G * SW), dtype=np_xdt)
        Xc[:, 0:V, :] = mc[a_rank].transpose(1, 2, 0).astype(np_xdt)
        Xc[:, V, :] = (lc[a_rank][None, :] <= trange[:, None] + 1).astype(np_xdt)
        Xc[:, V + 1, :] = 1.0
        Xc[:, 32 : 32 + V, :] = mc[b_rank].transpose(1, 2, 0).astype(np_xdt)
        Xc[:, 32 + V, :] = (lc[b_rank][None, :] <= trange[:, None] + 1).astype(np_xdt)
        Xc[:, 32 + V + 1, :] = 1.0
        if X_FP8:
            # DoubleRow layout [T-1, 32, G*2*SW]: per group, plane p (= x rows
            # 32p..32p+31) occupies the p-th SW-column block
            Xc = np.ascontiguousarray(
                Xc.reshape(T - 1, 2, 32, G, SW)
                .transpose(0, 2, 3, 1, 4)
                .reshape(T - 1, 32, G * 2 * SW)
            )
        in_maps.append({
            "X": Xc, "WXR": wxr_d, "WXZ": wxz_d, "WXN": wxn_d,
            "WHR": whr_d, "WHZ": whz_d, "WHN": whn_d, "IDN": idn_d,
            "W1A": w1a_d, "W1B": w1b_d, "W2T": w2_d,
            "BHN": bhn_d, "B1": b1_d, "B2H": b2_d, "H1F": h1_d,
        })

    nc = _build_nc(group_steps, dt)
    res = run_bass_kernel_spmd(nc, in_maps, core_ids=list(range(NCORES)), trace=TRACE)
    LAST_RESULT = res

    # OT column k -> local sorted rank
    k = np.arange(BC)
    sg_ = k // (4 * SW)
    si_ = (k % (4 * SW)) // (2 * SW)
    c_ = k % (2 * SW)
    g_ = 2 * sg_ + (c_ >= SW)
    L = g_ * 2 * SW + si_ * SW + (c_ % SW)

    out = np.empty((B, OUT), np.float32)
    for c in range(NCORES):
        out[core_idx[c][L]] = (res.results[c]["OT"].astype(np.float32).T + 1.0) * 0.5
    return out

